# revision 19
# baseline (speedup 1.0000x reference)
"""Trainium2 Bass kernel for BEV feature extraction (bilinear sampling).

Reference computation (CenterPoint-style):
  bev = transpose(spatial_features_2d, (0, 2, 3, 1))        # (B, H, W, C)
  x = (centers[..., 0] + 75.2) / 0.1 / 8                    # grid coords
  y = (centers[..., 1] + 75.2) / 0.1 / 8
  out[b, n, :] = bilinear_interpolate(bev[b], x[b, n], y[b, n])

Sharding: data-parallel over (batch, point-half) -> 8 cores, each handling
one batch element's 250-point half with the full 256-channel feature map.

Device kernel per core:
  - load 256 padded centers, compute grid coords / floor / weights on DVE
  - indirect-DMA gather: per point, two rows of 512 f32 from the NHWC
    feature map in HBM (pixel pair (y,x0),(y,x0+1) is contiguous)
  - blend with per-partition scalar FMAs (scalar_tensor_tensor)
  - store (256, 256) f32 result
"""

import os
import sys

import numpy as np

for _p in ("/opt/trn_rl_repo", "/root/.axon_site/_ro/trn_rl_repo"):
    if _p not in sys.path and os.path.isdir(_p):
        sys.path.insert(0, _p)

B, C, H, W = 4, 256, 188, 188
NPIX = H * W
N = 500
HALF = N // 2          # 250 real points per core
NPTS = 256             # padded to 2 x 128 partitions
N_CORES = 8

_CACHE = {}


def _build_bass():
    import concourse.bass as bass
    import concourse.tile as tile
    from concourse import mybir

    Alu = mybir.AluOpType
    f32 = mybir.dt.float32
    i32 = mybir.dt.int32

    nc = bass.Bass("TRN2", target_bir_lowering=False, debug=False)
    feats = nc.dram_tensor("feats", [NPIX, C], f32, kind="ExternalInput")
    ctrs = nc.dram_tensor("ctrs", [NPTS, 2], f32, kind="ExternalInput")
    out = nc.dram_tensor("out", [NPTS, C], f32, kind="ExternalOutput")

    with tile.TileContext(nc) as tc:
        with tc.tile_pool(name="p", bufs=1) as pool:
            # point n = p + 128*j lives at partition p, slot j
            ctr = pool.tile([128, 2, 2], f32)      # [p, j, (x, y)]
            nc.sync.dma_start(
                ctr[:, :, :], bass.AP(ctrs, 0, [[2, 128], [256, 2], [1, 2]])
            )

            # grid coords: (c + 75.2) * 1.25   (= /0.1/8)
            gxy = pool.tile([128, 2, 2], f32)
            nc.vector.tensor_scalar(
                out=gxy[:, :, :], in0=ctr[:, :, :],
                scalar1=75.2, scalar2=1.25, op0=Alu.add, op1=Alu.mult,
            )
            # floor: convert to int and back (any rounding mode), then
            # subtract 1 wherever the round went up. Coords are positive.
            xyi = pool.tile([128, 2, 2], i32)
            nc.vector.tensor_copy(xyi[:, :, :], gxy[:, :, :])
            xy0 = pool.tile([128, 2, 2], f32)
            nc.vector.tensor_copy(xy0[:, :, :], xyi[:, :, :])
            up = pool.tile([128, 2, 2], f32)
            nc.vector.tensor_tensor(
                up[:, :, :], xy0[:, :, :], gxy[:, :, :], Alu.is_gt
            )
            nc.vector.tensor_tensor(
                xy0[:, :, :], xy0[:, :, :], up[:, :, :], Alu.subtract
            )
            # pixel-row indices first, so the gathers launch while the
            # weight math still runs: iy0 = y0*W + x0, iy1 = iy0 + W
            idxf = pool.tile([128, 4], f32)
            nc.vector.scalar_tensor_tensor(
                out=idxf[:, 0:2], in0=xy0[:, :, 1], scalar=float(W),
                in1=xy0[:, :, 0], op0=Alu.mult, op1=Alu.add,
            )
            nc.vector.tensor_scalar(
                out=idxf[:, 2:4], in0=idxf[:, 0:2],
                scalar1=float(W), scalar2=None, op0=Alu.add,
            )
            idxi = pool.tile([128, 4], i32)
            nc.vector.tensor_copy(idxi[:, :], idxf[:, :])

            fxy = pool.tile([128, 2, 2], f32)
            nc.vector.tensor_tensor(
                fxy[:, :, :], gxy[:, :, :], xy0[:, :, :], Alu.subtract
            )
            # 1 - frac
            omf = pool.tile([128, 2, 2], f32)
            nc.vector.tensor_scalar(
                out=omf[:, :, :], in0=fxy[:, :, :],
                scalar1=1.0, scalar2=-1.0, op0=Alu.subtract, op1=Alu.mult,
            )

            # x-weight pair [1-fx, fx], then scale by y-weights:
            # wtop = [wa, wc] = [ (1-fx)(1-fy), fx(1-fy) ]   (y0 row)
            # wbot = [wb, wd] = [ (1-fx)fy,     fx*fy     ]  (y1 row)
            xw = pool.tile([128, 2, 2], f32)
            nc.vector.tensor_copy(xw[:, :, 0:1], omf[:, :, 0:1])
            nc.vector.tensor_copy(xw[:, :, 1:2], fxy[:, :, 0:1])
            wtop = pool.tile([128, 2, 2], f32)
            wbot = pool.tile([128, 2, 2], f32)
            nc.vector.tensor_tensor(
                wtop[:, :, :], xw[:, :, :],
                omf[:, :, 1:2].to_broadcast([128, 2, 2]), Alu.mult,
            )
            nc.vector.tensor_tensor(
                wbot[:, :, :], xw[:, :, :],
                fxy[:, :, 1:2].to_broadcast([128, 2, 2]), Alu.mult,
            )

            # gather: g[p, slot, :] = feats[idx[p, slot], :(2 pixels = 512)]
            # slots 0:2 = y0 rows, 2:4 = y1 rows; 0:256 = x0, 256:512 = x0+1.
            # One indirect DMA per idx column ([P,1]-idx / [P,D]-out is the
            # shape walrus compiles correctly). Order (0,2) before (1,3) so
            # the j=0 blend chain can start while j=1 rows are in flight.
            # Each gather gets a DVE observer copy that takes its DMA sem
            # wait, so the TensorScalarPtr blend ops (1-wait ISA capacity)
            # only ever need a DVE self-wait.
            g = pool.tile([128, 4, 512], f32)
            gobs = pool.tile([1, 4, 4], f32)
            obs_by_slot = {}
            for s in (0, 2, 1, 3):
                nc.gpsimd.indirect_dma_start(
                    out=g[:, s, :],
                    out_offset=None,
                    in_=feats.ap(),
                    in_offset=bass.IndirectOffsetOnAxis(
                        ap=idxi[:, s : s + 1], axis=0
                    ),
                )
                obs_by_slot[s] = nc.vector.tensor_copy(
                    gobs[:, s, :], g[0:1, s, 0:4]
                )

            # blend: res[p,j,:] = wa*g(y0,x0) + wc*g(y0,x1) + wb*g(y1,x0) + wd*g(y1,x1)
            # Each half stores as soon as its chain finishes, on separate
            # HWDGE queues (SP / ACT) so the stores overlap. A DVE observer
            # takes store0's sem so the final drain keeps a single wait.
            res = pool.tile([128, 2, 256], f32)
            stores = []
            for j in range(2):
                first = nc.vector.tensor_scalar_mul(
                    res[:, j, :], g[:, j, 0:256], wtop[:, j, 0:1]
                )
                tile.add_dep_helper(first.ins, obs_by_slot[j].ins, reason="obs")
                for src, wsc, dep in (
                    (g[:, j, 256:512], wtop[:, j, 1:2], j),
                    (g[:, 2 + j, 0:256], wbot[:, j, 0:1], 2 + j),
                    (g[:, 2 + j, 256:512], wbot[:, j, 1:2], 2 + j),
                ):
                    fma = nc.vector.scalar_tensor_tensor(
                        out=res[:, j, :], in0=src, scalar=wsc, in1=res[:, j, :],
                        op0=Alu.mult, op1=Alu.add,
                    )
                    tile.add_dep_helper(fma.ins, obs_by_slot[dep].ins, reason="obs")
                eng = nc.sync if j == 0 else nc.scalar
                stores.append(
                    eng.dma_start(
                        bass.AP(out, j * 128 * C, [[C, 128], [1, C]]),
                        res[:, j, :],
                    )
                )
            sobs = pool.tile([1, 4], f32)
            so = nc.vector.tensor_copy(sobs[:, :], res[0:1, 0, 0:4])
            tile.add_dep_helper(so.ins, stores[0].ins, reason="obs store0")

    _prune_final_drain_waits(nc, mybir)
    return nc


_ENGINE_SEM_PREFIXES = ("DVE", "Activation", "Pool", "PE", "SP", "Sync")


def _prune_final_drain_waits(nc, mybir, max_drain_waits=1):
    """Walrus on this path allows only ONE sync-wait command per instruction.

    Two classes of redundant waits Tile emits:
    1. Self-engine waits (a DVE op waiting on the DVE proc sem): implied by
       in-order execution of the engine's instruction stream.
    2. The final pre-barrier drain waits on every outstanding proc sem. The
       all-engine barrier that follows already guarantees engine completion,
       and any DMA sem some engine instruction waited on is transitively
       complete. Only sink DMA sems (observed by nobody — the output store)
       must remain.
    """
    eng_prefix = {
        mybir.EngineType.DVE: "DVE",
        mybir.EngineType.Activation: "Activation",
        mybir.EngineType.Pool: "Pool",
        mybir.EngineType.PE: "PE",
        mybir.EngineType.SP: "SP",
    }
    observed = {}  # sem name -> max value waited on by any engine instruction
    drains = []
    for inst in nc.inst_map.values():
        if not inst.sync_info or not inst.sync_info.on_wait:
            continue
        if isinstance(inst, mybir.InstDrain):
            drains.append(inst)
            continue
        for w in inst.sync_info.on_wait:
            if w.wait_mode == "sem-ge-imm" and w.wait_value is not None:
                observed[w.ant_name] = max(
                    observed.get(w.ant_name, 0), w.wait_value
                )

    # regular instructions must already be single-wait (the kernel is
    # structured with observer ops so Tile never needs a multi-wait join)
    for inst in nc.inst_map.values():
        if (
            isinstance(inst, mybir.InstDrain)
            or not inst.sync_info
            or not inst.sync_info.on_wait
        ):
            continue
        assert len(inst.sync_info.on_wait) <= 1, (
            f"{inst.name} ({type(inst).__name__}): multi-wait: "
            f"{[(w.ant_name, w.wait_value) for w in inst.sync_info.on_wait]}"
        )

    # final drain: keep only sink DMA sems
    for inst in drains:
        waits = inst.sync_info.on_wait
        if not any(w.wait_mode == "sem-ge-imm" for w in waits):
            continue  # barrier drains (sem-eq) untouched
        kept = [
            w
            for w in waits
            if not w.ant_name.startswith(_ENGINE_SEM_PREFIXES)
            and observed.get(w.ant_name, -1) < (w.wait_value or 0)
        ]
        inst.sync_info.on_wait = kept
        assert len(kept) <= max_drain_waits, (
            f"final drain still has {len(kept)} waits: "
            f"{[(w.ant_name, w.wait_value) for w in kept]}"
        )


def get_nc():
    if "nc" not in _CACHE:
        _CACHE["nc"] = _build_bass()
    return _CACHE["nc"]


def _shard_inputs(spatial_features_2d, centers):
    """Full inputs -> per-core in_maps (host-side layout prep only)."""
    feats = np.ascontiguousarray(
        np.transpose(np.asarray(spatial_features_2d, dtype=np.float32), (0, 2, 3, 1))
    ).reshape(B, NPIX, C)
    ctr = np.asarray(centers, dtype=np.float32)[:, :, :2]
    in_maps = []
    for k in range(N_CORES):
        b, h = k // 2, k % 2
        c_k = np.zeros((NPTS, 2), dtype=np.float32)
        c_k[:HALF] = ctr[b, h * HALF : (h + 1) * HALF]
        in_maps.append({"feats": feats[b], "ctrs": c_k})
    return in_maps


def _unshard(results):
    out = np.empty((B, N, C), dtype=np.float32)
    for k in range(N_CORES):
        b, h = k // 2, k % 2
        out[b, h * HALF : (h + 1) * HALF] = results[k]["out"][:HALF]
    return out


def run(spatial_features_2d, centers, trace=False):
    from concourse.bass_utils import run_bass_kernel_spmd

    in_maps = _shard_inputs(spatial_features_2d, centers)
    res = run_bass_kernel_spmd(
        get_nc(), in_maps, core_ids=list(range(N_CORES)), trace=trace
    )
    return _unshard(res.results), res


def kernel(spatial_features_2d, centers):
    out, _ = run(spatial_features_2d, centers)
    return out
